# revision 13
# baseline (speedup 1.0000x reference)
"""Trainium2 Bass kernel for nn_Decoder (RBF decoder).

Math (shapes: t (4,512,1), z (4,512,128), x (4,512,1), sigma (128,),
W (2,128), b (2,)):
    diff[b,n,m] = x[b,m] - t[b,n]                  (XD=1, sum(-1) trivial)
    K[b,n,m,c]  = exp(-0.5 * (diff/exp(sigma[c]))^2)
    y[b,m,c]    = sum_n z[b,n,c] * K[b,n,m,c]
    out[b,m,:]  = y[b,m,:] @ W.T + b

When all sigma[c] are equal (they are zeros for this problem), K is
channel-independent, so W folds into z up front:
    zw[b] = z[b] @ W.T              (host, (N,2) per batch -- tiny)
    out[b].T = sum_n zw[b,n,:]^T G[b][n,:],  G = exp(s*(x_m - t_n)^2),
    s = -0.5*exp(-2*sigma).

Device mapping (8 cores, SPMD): core k handles batch b=k//2, n-half
h=k%2 (256 grid points = 2 n-tiles of 128 partitions). The exponent
E[n,m] = s*x_m^2 + s*t_n^2 - 2s*x_m*t_n is rank-3 over (n,m), so it is
built directly in PSUM by one K=3 matmul per n-tile from row-packed
host data -- no (128,M) x-broadcast DMA and no Square op at all:
    rhs  (3,512): [s*x^2 ; 1 ; -2s*x]          (cols = m)
    lhsT (3,128): [1 ; s*t^2 ; t]   per n-tile (cols = n partition)
All device inputs ride in ONE (4,1024) f32 DMA (4 descriptors x 4KB):
cols 0-511 rhs rows, 512-767 the two lhsT blocks, 768-895 zw^T rows
(4,128), 896-899 a 4x4 identity. zw^T is transposed on-chip by a tiny
matmul against the identity (out (128,4) PSUM), evicted to bf16 SBUF.
Per n-tile: Exp (ScalarE, PSUM src -> bf16 SBUF), then the G-matmul
(bf16, lhsT=zw cols) accumulates into a (2,512) PSUM out tile. DVE
evicts it and one DMA returns out[b].T partial; host sums the two
n-half partials, transposes, adds bias.

E-matmuls use fp32r (1 cycle/row vs fp32's 4 when free dim >= 256);
exponent abs err ~5e-4 -> far inside the 2e-2 gate. A short string of
bf16 warm-up matmuls on a zeroed scratch tile runs during the
DMA-wait window to push the PE p-state up, and a dummy activation
pulls the ~1.3us ACT_TABLE_LOAD off the critical path.

s enters only through the host-packed data (nothing baked into the
NEFF), so one compiled module serves any sigma; non-uniform sigma
falls back to grouping channels by unique value and summing group
outputs (exact -- the output is linear in z).

Sync-wait discipline: this container's walrus allows a single on_wait
per instruction, so _split_multi_waits rewrites the scheduled BIR,
hoisting extra waits onto same-engine NOPs placed immediately before
the instruction (same-engine program order preserves semantics).
"""

import numpy as np

B, N, M, C, Y = 4, 512, 512, 128, 2
NHALF = N // 2  # n-slice per core
NT = NHALF // 128  # n-tiles of 128 per core
PKW = 1024  # packed input tile free width

N_WARM = 4

_CACHE = {}


def _split_multi_waits(nc):
    import concourse.mybir as mybir

    for fn in nc.m.functions:
        for blk in fn.blocks:
            il = blk.instructions
            new = []
            for inst in il:
                si = inst.sync_info
                if si is not None and si.on_wait is not None and len(si.on_wait) > 1:
                    waits = list(si.on_wait)
                    for j, w in enumerate(waits[:-1]):
                        new.append(
                            mybir.InstNoOp(
                                name=f"{inst.name}-w{j}",
                                engine=inst.engine,
                                sync_info=mybir.SyncInfo(on_wait=[w], on_update=[]),
                                bass_nofuse=True,
                            )
                        )
                    si.on_wait = [waits[-1]]
                    inst.sync_info = si
                new.append(inst)
            il[:] = new


def build_bass():
    import concourse.bass as bass
    import concourse.mybir as mybir
    import concourse.tile as tile

    f32 = mybir.dt.float32
    bf16 = mybir.dt.bfloat16
    nc = bass.Bass()
    pk = nc.dram_tensor("pk", (4, PKW), bf16, kind="ExternalInput")
    o = nc.dram_tensor("o", (Y, M), f32, kind="ExternalOutput")

    from contextlib import ExitStack

    with ExitStack() as stk:
        dma_sem = stk.enter_context(nc.semaphore())
        pk_raw = stk.enter_context(nc.sbuf_tensor((4, PKW), bf16))
        # Input DMA issued BEFORE the TileContext pool-entry barrier: the SP
        # engine fires it as soon as its register preamble is done instead of
        # waiting for all engines to drain into the pool barrier (~0.8us).
        # PE is the only consumer of pk_raw; one manual wait_ge covers it.
        nc.sync.dma_start(out=pk_raw[:], in_=pk[:]).then_inc(dma_sem, 16)

        with tile.TileContext(nc) as tc:
            with (
                tc.tile_pool(name="sb", bufs=1) as sb,
                tc.tile_pool(name="ps", bufs=1, space="PSUM") as ps,
            ):
                # PE warm-up on a zeroed bf16 scratch + dummy activation to
                # pull the ACT_TABLE_LOAD into the DMA-wait window.
                scr = sb.tile([128, M], bf16)
                nc.vector.memset(scr, 0.0)
                warm_act = sb.tile([128, 1], f32)
                nc.scalar.activation(
                    warm_act, scr[:, 0:1], mybir.ActivationFunctionType.Exp
                )
                w_ps = ps.tile([128, M], f32)
                for _ in range(N_WARM):
                    nc.tensor.matmul(
                        w_ps, lhsT=scr[:, 0:128], rhs=scr, start=True, stop=True
                    )

                # zw^T (4,128) -> (128,4) PSUM via matmul against the packed
                # I4, evicted to bf16 SBUF for the G-matmul lhsT. The wait on
                # the manual DMA semaphore is attached POST-scheduling (the
                # tile sim cannot see the pre-context DMA and would deadlock
                # on an in-context wait_ge).
                zw_ps = ps.tile([128, 4], f32)
                pk_consumers = []
                pk_consumers.append(
                    nc.tensor.matmul(
                        zw_ps,
                        lhsT=pk_raw[0:4, 768:896],
                        rhs=pk_raw[0:4, 896:900],
                        start=True,
                        stop=True,
                    )
                )
                zw_sb = sb.tile([128, 4], bf16)
                nc.vector.tensor_copy(zw_sb, zw_ps)

                # Per n-tile: E = s*(x-t)^2 via K=3 bf16 matmul, exp on
                # ScalarE (PSUM src -> bf16 SBUF), G-matmul accumulate.
                o_ps = ps.tile([Y, M], f32)
                e_tiles = []
                for nt in range(NT):
                    e_ps = ps.tile([128, M], f32, tag=f"e{nt}")
                    pk_consumers.append(
                        nc.tensor.matmul(
                            e_ps,
                            lhsT=pk_raw[0:3, 512 + nt * 128 : 512 + (nt + 1) * 128],
                            rhs=pk_raw[0:3, 0:512],
                            start=True,
                            stop=True,
                        )
                    )
                    e_tiles.append(e_ps)
                for nt in range(NT):
                    k_sb = sb.tile([128, M], bf16, tag=f"k{nt}")
                    nc.scalar.activation(
                        k_sb, e_tiles[nt], mybir.ActivationFunctionType.Exp
                    )
                    if nt == 0:
                        # Filler matmul keeps the PE p-state ramp alive
                        # through the exp wait.
                        nc.tensor.matmul(
                            w_ps, lhsT=scr[:, 0:128], rhs=scr, start=True, stop=True
                        )
                    nc.tensor.matmul(
                        o_ps,
                        lhsT=zw_sb[:, nt * Y : (nt + 1) * Y],
                        rhs=k_sb,
                        start=(nt == 0),
                        stop=(nt == NT - 1),
                    )
                o_sb = sb.tile([Y, M], f32)
                nc.vector.tensor_copy(o_sb, o_ps)
                nc.sync.dma_start(out=o[:], in_=o_sb)
        # Attach the input-DMA wait to every PE consumer of pk_raw, now that
        # the tile scheduler has run (it may have reordered them); later
        # waits are trivially satisfied.
        for mm in pk_consumers:
            mm.wait_op(dma_sem, 16, "sem-ge")
    _split_multi_waits(nc)
    return nc


def _get_nc():
    if "nc" not in _CACHE:
        _CACHE["nc"] = build_bass()
    return _CACHE["nc"]


def _in_maps_for_group(t, x, zw, s):
    """Build the 8 per-core input dicts for one sigma-group.

    zw: (B, N, Y) = z[:, :, group] @ W[:, group].T
    """
    import ml_dtypes

    in_maps = []
    for core in range(8):
        b, h = core // 2, core % 2
        xb = x[b, :, 0]
        pkm = np.zeros((4, PKW), np.float32)
        pkm[0, 0:512] = s * xb * xb
        pkm[1, 0:512] = 1.0
        pkm[2, 0:512] = -2.0 * s * xb
        for nt in range(NT):
            tb = t[b, h * NHALF + nt * 128 : h * NHALF + (nt + 1) * 128, 0]
            c0 = 512 + nt * 128
            pkm[0, c0 : c0 + 128] = 1.0
            pkm[1, c0 : c0 + 128] = s * tb * tb
            pkm[2, c0 : c0 + 128] = tb
            lo = h * NHALF + nt * 128
            pkm[nt * Y, 768:896][0:128] = zw[b, lo : lo + 128, 0]
            pkm[nt * Y + 1, 768:896][0:128] = zw[b, lo : lo + 128, 1]
        pkm[0:4, 896:900] = np.eye(4, dtype=np.float32)
        in_maps.append({"pk": pkm.astype(ml_dtypes.bfloat16)})
    return in_maps


def _run_group(t, x, zw, s, trace=False):
    from concourse.bass_utils import run_bass_kernel_spmd

    res = run_bass_kernel_spmd(
        _get_nc(),
        _in_maps_for_group(t, x, zw, s),
        core_ids=list(range(8)),
        trace=trace,
    )
    out = np.zeros((B, M, Y), np.float32)
    for b in range(B):
        acc = res.results[2 * b]["o"] + res.results[2 * b + 1]["o"]  # (Y, M)
        out[b] = acc.T
    return out, res


def kernel(**inputs):
    t = np.asarray(inputs["t"], np.float32)
    z = np.asarray(inputs["z"], np.float32)
    x = np.asarray(inputs["x"], np.float32)
    sigma = np.asarray(inputs["sigma"], np.float32)
    W = np.asarray(inputs["W"], np.float32)
    bias = np.asarray(inputs["b"], np.float32)

    trace = bool(_CACHE.pop("trace", False))
    out = np.zeros((B, M, Y), np.float32)
    if np.all(sigma == sigma[0]):
        s = -0.5 * float(np.exp(-2.0 * sigma[0]))
        zw = z @ W.T  # (B, N, Y)
        grp_out, res = _run_group(t, x, zw.astype(np.float32), s, trace=trace)
        out += grp_out
        _CACHE["last_results"] = res
    else:
        for val in np.unique(sigma):
            idx = np.nonzero(sigma == val)[0]
            zw = z[:, :, idx] @ W[:, idx].T
            s = -0.5 * float(np.exp(-2.0 * val))
            grp_out, res = _run_group(t, x, zw.astype(np.float32), s, trace=False)
            out += grp_out
    out += bias[None, None, :]
    return out


# revision 17
# speedup vs baseline: 1.0564x; 1.0564x over previous
"""Trainium2 Bass kernel for nn_Decoder (RBF decoder).

Math (shapes: t (4,512,1), z (4,512,128), x (4,512,1), sigma (128,),
W (2,128), b (2,)):
    diff[b,n,m] = x[b,m] - t[b,n]                  (XD=1, sum(-1) trivial)
    K[b,n,m,c]  = exp(-0.5 * (diff/exp(sigma[c]))^2)
    y[b,m,c]    = sum_n z[b,n,c] * K[b,n,m,c]
    out[b,m,:]  = y[b,m,:] @ W.T + b

When all sigma[c] are equal (they are zeros for this problem), K is
channel-independent, so W folds into z up front:
    zw[b] = z[b] @ W.T              (host, (N,2) per batch -- tiny)
    out[b].T = sum_n zw[b,n,:]^T G[b][n,:],  G = exp(s*(x_m - t_n)^2),
    s = -0.5*exp(-2*sigma).

Device mapping (8 cores, SPMD): core k handles batch b=k//2, n-half
h=k%2 (256 grid points = 2 n-tiles of 128 partitions). The exponent
E[n,m] = s*x_m^2 + s*t_n^2 - 2s*x_m*t_n is rank-3 over (n,m), so it is
built directly in PSUM by one K=3 matmul per n-tile from row-packed
host data -- no (128,M) x-broadcast DMA and no Square op at all:
    rhs  (3,512): [s*x^2 ; 1 ; -2s*x]          (cols = m)
    lhsT (3,128): [1 ; s*t^2 ; t]   per n-tile (cols = n partition)
All device inputs ride in ONE (4,1024) f32 DMA (4 descriptors x 4KB):
cols 0-511 rhs rows, 512-767 the two lhsT blocks, 768-895 zw^T rows
(4,128), 896-899 a 4x4 identity. zw^T is transposed on-chip by a tiny
matmul against the identity (out (128,4) PSUM), evicted to bf16 SBUF.
Per n-tile: Exp (ScalarE, PSUM src -> bf16 SBUF), then the G-matmul
(bf16, lhsT=zw cols) accumulates into a (2,512) PSUM out tile. DVE
evicts it and one DMA returns out[b].T partial; host sums the two
n-half partials, transposes, adds bias.

E-matmuls use fp32r (1 cycle/row vs fp32's 4 when free dim >= 256);
exponent abs err ~5e-4 -> far inside the 2e-2 gate. A short string of
bf16 warm-up matmuls on a zeroed scratch tile runs during the
DMA-wait window to push the PE p-state up, and a dummy activation
pulls the ~1.3us ACT_TABLE_LOAD off the critical path.

s enters only through the host-packed data (nothing baked into the
NEFF), so one compiled module serves any sigma; non-uniform sigma
falls back to grouping channels by unique value and summing group
outputs (exact -- the output is linear in z).

Sync-wait discipline: this container's walrus allows a single on_wait
per instruction, so _split_multi_waits rewrites the scheduled BIR,
hoisting extra waits onto same-engine NOPs placed immediately before
the instruction (same-engine program order preserves semantics).
"""

import numpy as np

B, N, M, C, Y = 4, 512, 512, 128, 2
NHALF = N // 2  # n-slice per core
NT = NHALF // 128  # n-tiles of 128 per core
PKW = 1024  # packed input tile free width

N_WARM = 4
# Scheduler hint: sim-time (ms) at which the pre-context input DMA's data
# lands in SBUF, measured from the start of the scheduled body.
DATA_READY_MS = 0.002

_CACHE = {}


def _split_multi_waits(nc):
    import concourse.mybir as mybir

    for fn in nc.m.functions:
        for blk in fn.blocks:
            il = blk.instructions
            new = []
            for inst in il:
                si = inst.sync_info
                if si is not None and si.on_wait is not None and len(si.on_wait) > 1:
                    waits = list(si.on_wait)
                    for j, w in enumerate(waits[:-1]):
                        new.append(
                            mybir.InstNoOp(
                                name=f"{inst.name}-w{j}",
                                engine=inst.engine,
                                sync_info=mybir.SyncInfo(on_wait=[w], on_update=[]),
                                bass_nofuse=True,
                            )
                        )
                    si.on_wait = [waits[-1]]
                    inst.sync_info = si
                new.append(inst)
            il[:] = new


def build_bass():
    import concourse.bass as bass
    import concourse.mybir as mybir
    import concourse.tile as tile

    f32 = mybir.dt.float32
    bf16 = mybir.dt.bfloat16
    nc = bass.Bass()
    pk = nc.dram_tensor("pk", (4, PKW), bf16, kind="ExternalInput")
    o = nc.dram_tensor("o", (Y, M), f32, kind="ExternalOutput")

    from contextlib import ExitStack

    with ExitStack() as stk:
        dma_sem = stk.enter_context(nc.semaphore())
        pk_raw = stk.enter_context(nc.sbuf_tensor((4, PKW), bf16))
        # Input DMA issued BEFORE the TileContext pool-entry barrier: the SP
        # engine fires it as soon as its register preamble is done instead of
        # waiting for all engines to drain into the pool barrier (~0.8us).
        # PE is the only consumer of pk_raw; one manual wait_ge covers it.
        nc.sync.dma_start(out=pk_raw[:], in_=pk[:]).then_inc(dma_sem, 16)

        with tile.TileContext(nc) as tc:
            with (
                tc.tile_pool(name="sb", bufs=1) as sb,
                tc.tile_pool(name="ps", bufs=1, space="PSUM") as ps,
            ):
                # PE warm-up on a zeroed bf16 scratch + dummy activation to
                # pull the ACT_TABLE_LOAD into the DMA-wait window.
                scr = sb.tile([128, M], bf16)
                nc.vector.memset(scr, 0.0)
                warm_act = sb.tile([128, 1], f32)
                nc.scalar.activation(
                    warm_act, scr[:, 0:1], mybir.ActivationFunctionType.Exp
                )
                w_ps = ps.tile([128, M], f32)
                for _ in range(N_WARM):
                    nc.tensor.matmul(
                        w_ps, lhsT=scr[:, 0:128], rhs=scr, start=True, stop=True
                    )

                # zw^T (4,128) -> (128,4) PSUM via matmul against the packed
                # I4, evicted to bf16 SBUF for the G-matmul lhsT. The wait on
                # the manual DMA semaphore is attached POST-scheduling (the
                # tile sim cannot see the pre-context DMA and would deadlock
                # on an in-context wait_ge). tile_wait_until tells the
                # scheduler pk_raw data lands ~2us into the body so it packs
                # the warm-ups BEFORE the consumers, not between them.
                zw_ps = ps.tile([128, 4], f32)
                pk_consumers = []
                with tc.tile_wait_until(DATA_READY_MS):
                    pk_consumers.append(
                        nc.tensor.matmul(
                            zw_ps,
                            lhsT=pk_raw[0:4, 768:896],
                            rhs=pk_raw[0:4, 896:900],
                            start=True,
                            stop=True,
                        )
                    )
                zw_sb = sb.tile([128, 4], bf16)
                nc.vector.tensor_copy(zw_sb, zw_ps)

                # Per n-tile: E = s*(x-t)^2 via K=3 bf16 matmul, exp on
                # ScalarE (PSUM src -> bf16 SBUF), G-matmul accumulate.
                o_ps = ps.tile([Y, M], f32)
                e_tiles = []
                for nt in range(NT):
                    e_ps = ps.tile([128, M], f32, tag=f"e{nt}")
                    with tc.tile_wait_until(DATA_READY_MS):
                        pk_consumers.append(
                            nc.tensor.matmul(
                                e_ps,
                                lhsT=pk_raw[
                                    0:3, 512 + nt * 128 : 512 + (nt + 1) * 128
                                ],
                                rhs=pk_raw[0:3, 0:512],
                                start=True,
                                stop=True,
                            )
                        )
                    e_tiles.append(e_ps)
                for nt in range(NT):
                    k_sb = sb.tile([128, M], bf16, tag=f"k{nt}")
                    nc.scalar.activation(
                        k_sb, e_tiles[nt], mybir.ActivationFunctionType.Exp
                    )
                    if nt == 0:
                        # Filler matmul keeps the PE p-state ramp alive
                        # through the exp wait.
                        nc.tensor.matmul(
                            w_ps, lhsT=scr[:, 0:128], rhs=scr, start=True, stop=True
                        )
                    nc.tensor.matmul(
                        o_ps,
                        lhsT=zw_sb[:, nt * Y : (nt + 1) * Y],
                        rhs=k_sb,
                        start=(nt == 0),
                        stop=(nt == NT - 1),
                    )
                o_sb = sb.tile([Y, M], f32)
                nc.vector.tensor_copy(o_sb, o_ps)
                nc.sync.dma_start(out=o[:], in_=o_sb)
        # Attach the input-DMA wait to every PE consumer of pk_raw, now that
        # the tile scheduler has run (it may have reordered them); later
        # waits are trivially satisfied.
        for mm in pk_consumers:
            mm.wait_op(dma_sem, 16, "sem-ge")
    _split_multi_waits(nc)
    return nc


def _get_nc():
    if "nc" not in _CACHE:
        _CACHE["nc"] = build_bass()
    return _CACHE["nc"]


def _in_maps_for_group(t, x, zw, s):
    """Build the 8 per-core input dicts for one sigma-group.

    zw: (B, N, Y) = z[:, :, group] @ W[:, group].T
    """
    import ml_dtypes

    in_maps = []
    for core in range(8):
        b, h = core // 2, core % 2
        xb = x[b, :, 0]
        pkm = np.zeros((4, PKW), np.float32)
        pkm[0, 0:512] = s * xb * xb
        pkm[1, 0:512] = 1.0
        pkm[2, 0:512] = -2.0 * s * xb
        for nt in range(NT):
            tb = t[b, h * NHALF + nt * 128 : h * NHALF + (nt + 1) * 128, 0]
            c0 = 512 + nt * 128
            pkm[0, c0 : c0 + 128] = 1.0
            pkm[1, c0 : c0 + 128] = s * tb * tb
            pkm[2, c0 : c0 + 128] = tb
            lo = h * NHALF + nt * 128
            pkm[nt * Y, 768:896][0:128] = zw[b, lo : lo + 128, 0]
            pkm[nt * Y + 1, 768:896][0:128] = zw[b, lo : lo + 128, 1]
        pkm[0:4, 896:900] = np.eye(4, dtype=np.float32)
        in_maps.append({"pk": pkm.astype(ml_dtypes.bfloat16)})
    return in_maps


def _run_group(t, x, zw, s, trace=False):
    from concourse.bass_utils import run_bass_kernel_spmd

    res = run_bass_kernel_spmd(
        _get_nc(),
        _in_maps_for_group(t, x, zw, s),
        core_ids=list(range(8)),
        trace=trace,
    )
    out = np.zeros((B, M, Y), np.float32)
    for b in range(B):
        acc = res.results[2 * b]["o"] + res.results[2 * b + 1]["o"]  # (Y, M)
        out[b] = acc.T
    return out, res


def kernel(**inputs):
    t = np.asarray(inputs["t"], np.float32)
    z = np.asarray(inputs["z"], np.float32)
    x = np.asarray(inputs["x"], np.float32)
    sigma = np.asarray(inputs["sigma"], np.float32)
    W = np.asarray(inputs["W"], np.float32)
    bias = np.asarray(inputs["b"], np.float32)

    trace = bool(_CACHE.pop("trace", False))
    out = np.zeros((B, M, Y), np.float32)
    if np.all(sigma == sigma[0]):
        s = -0.5 * float(np.exp(-2.0 * sigma[0]))
        zw = z @ W.T  # (B, N, Y)
        grp_out, res = _run_group(t, x, zw.astype(np.float32), s, trace=trace)
        out += grp_out
        _CACHE["last_results"] = res
    else:
        for val in np.unique(sigma):
            idx = np.nonzero(sigma == val)[0]
            zw = z[:, :, idx] @ W[:, idx].T
            s = -0.5 * float(np.exp(-2.0 * val))
            grp_out, res = _run_group(t, x, zw.astype(np.float32), s, trace=False)
            out += grp_out
    out += bias[None, None, :]
    return out


# revision 21
# speedup vs baseline: 1.0732x; 1.0160x over previous
"""Trainium2 Bass kernel for nn_Decoder (RBF decoder).

Math (shapes: t (4,512,1), z (4,512,128), x (4,512,1), sigma (128,),
W (2,128), b (2,)):
    diff[b,n,m] = x[b,m] - t[b,n]                  (XD=1, sum(-1) trivial)
    K[b,n,m,c]  = exp(-0.5 * (diff/exp(sigma[c]))^2)
    y[b,m,c]    = sum_n z[b,n,c] * K[b,n,m,c]
    out[b,m,:]  = y[b,m,:] @ W.T + b

When all sigma[c] are equal (they are zeros for this problem), K is
channel-independent, so W folds into z up front:
    zw[b] = z[b] @ W.T              (host, (N,2) per batch -- tiny)
    out[b].T = sum_n zw[b,n,:]^T G[b][n,:],  G = exp(s*(x_m - t_n)^2),
    s = -0.5*exp(-2*sigma).

Device mapping (8 cores, SPMD): core k handles batch b=k//2, n-half
h=k%2 (256 grid points = 2 n-tiles of 128 partitions). The exponent
E[n,m] = s*x_m^2 + s*t_n^2 - 2s*x_m*t_n is rank-3 over (n,m), so it is
built directly in PSUM by one K=3 matmul per n-tile from row-packed
host data -- no (128,M) x-broadcast DMA and no Square op at all:
    rhs  (3,512): [s*x^2 ; 1 ; -2s*x]          (cols = m)
    lhsT (3,128): [1 ; s*t^2 ; t]   per n-tile (cols = n partition)
All device inputs ride in ONE (4,1024) f32 DMA (4 descriptors x 4KB):
cols 0-511 rhs rows, 512-767 the two lhsT blocks, 768-895 zw^T rows
(4,128), 896-899 a 4x4 identity. zw^T is transposed on-chip by a tiny
matmul against the identity (out (128,4) PSUM), evicted to bf16 SBUF.
Per n-tile: Exp (ScalarE, PSUM src -> bf16 SBUF), then the G-matmul
(bf16, lhsT=zw cols) accumulates into a (2,512) PSUM out tile. DVE
evicts it and one DMA returns out[b].T partial; host sums the two
n-half partials, transposes, adds bias.

E-matmuls use fp32r (1 cycle/row vs fp32's 4 when free dim >= 256);
exponent abs err ~5e-4 -> far inside the 2e-2 gate. A short string of
bf16 warm-up matmuls on a zeroed scratch tile runs during the
DMA-wait window to push the PE p-state up, and a dummy activation
pulls the ~1.3us ACT_TABLE_LOAD off the critical path.

s enters only through the host-packed data (nothing baked into the
NEFF), so one compiled module serves any sigma; non-uniform sigma
falls back to grouping channels by unique value and summing group
outputs (exact -- the output is linear in z).

Sync-wait discipline: this container's walrus allows a single on_wait
per instruction, so _split_multi_waits rewrites the scheduled BIR,
hoisting extra waits onto same-engine NOPs placed immediately before
the instruction (same-engine program order preserves semantics).
"""

import numpy as np

B, N, M, C, Y = 4, 512, 512, 128, 2
NHALF = N // 2  # n-slice per core
NT = NHALF // 128  # n-tiles of 128 per core
PKW = 1024  # packed input tile free width

N_WARM = 3
# Scheduler hint: sim-time (ms) at which the pre-context input DMA's data
# lands in SBUF, measured from the start of the scheduled body.
DATA_READY_MS = 0.0022

_CACHE = {}


def _split_multi_waits(nc):
    import concourse.mybir as mybir

    for fn in nc.m.functions:
        for blk in fn.blocks:
            il = blk.instructions
            new = []
            for inst in il:
                si = inst.sync_info
                if si is not None and si.on_wait is not None and len(si.on_wait) > 1:
                    waits = list(si.on_wait)
                    for j, w in enumerate(waits[:-1]):
                        new.append(
                            mybir.InstNoOp(
                                name=f"{inst.name}-w{j}",
                                engine=inst.engine,
                                sync_info=mybir.SyncInfo(on_wait=[w], on_update=[]),
                                bass_nofuse=True,
                            )
                        )
                    si.on_wait = [waits[-1]]
                    inst.sync_info = si
                new.append(inst)
            il[:] = new


def build_bass():
    import concourse.bass as bass
    import concourse.mybir as mybir
    import concourse.tile as tile

    f32 = mybir.dt.float32
    bf16 = mybir.dt.bfloat16
    nc = bass.Bass()
    pk = nc.dram_tensor("pk", (4, PKW), bf16, kind="ExternalInput")
    o = nc.dram_tensor("o", (Y, M), f32, kind="ExternalOutput")

    from contextlib import ExitStack

    with ExitStack() as stk:
        dma_sem = stk.enter_context(nc.semaphore())
        pk_raw = stk.enter_context(nc.sbuf_tensor((4, PKW), bf16))
        # Input DMA issued BEFORE the TileContext pool-entry barrier: the SP
        # engine fires it as soon as its register preamble is done instead of
        # waiting for all engines to drain into the pool barrier (~0.8us).
        # PE is the only consumer of pk_raw; one manual wait_ge covers it.
        nc.sync.dma_start(out=pk_raw[:], in_=pk[:]).then_inc(dma_sem, 16)

        with tile.TileContext(nc) as tc:
            with (
                tc.tile_pool(name="sb", bufs=1) as sb,
                tc.tile_pool(name="ps", bufs=1, space="PSUM") as ps,
            ):
                # PE warm-up on a zeroed bf16 scratch + dummy activation to
                # pull the ACT_TABLE_LOAD into the DMA-wait window. The
                # zeros column z1 doubles as the activation bias so the
                # framework's const-AP memsets (which delay the pool-entry
                # barrier on GpSimd) are never emitted.
                scr = sb.tile([128, M], bf16)
                nc.gpsimd.memset(scr, 0.0)
                z1 = sb.tile([128, 1], f32)
                nc.vector.memset(z1, 0.0)
                warm_act = sb.tile([128, 1], f32)
                nc.scalar.activation(
                    warm_act,
                    scr[:, 0:1],
                    mybir.ActivationFunctionType.Exp,
                    bias=z1[:, 0:1],
                )
                w_ps = ps.tile([128, M], f32)
                for _ in range(N_WARM):
                    nc.tensor.matmul(
                        w_ps, lhsT=scr[:, 0:128], rhs=scr, start=True, stop=True
                    )

                # zw^T (4,128) -> (128,4) PSUM via matmul against the packed
                # I4, evicted to bf16 SBUF for the G-matmul lhsT. The wait on
                # the manual DMA semaphore is attached POST-scheduling (the
                # tile sim cannot see the pre-context DMA and would deadlock
                # on an in-context wait_ge). tile_wait_until tells the
                # scheduler pk_raw data lands ~2us into the body so it packs
                # the warm-ups BEFORE the consumers, not between them.
                zw_ps = ps.tile([128, 4], f32)
                pk_consumers = []
                with tc.tile_wait_until(DATA_READY_MS):
                    pk_consumers.append(
                        nc.tensor.matmul(
                            zw_ps,
                            lhsT=pk_raw[0:4, 768:896],
                            rhs=pk_raw[0:4, 896:900],
                            start=True,
                            stop=True,
                        )
                    )
                zw_sb = sb.tile([128, 4], bf16)
                nc.vector.tensor_copy(zw_sb, zw_ps)

                # Per n-tile: E = s*(x-t)^2 via K=3 bf16 matmul, exp on
                # ScalarE (PSUM src -> bf16 SBUF), G-matmul accumulate.
                o_ps = ps.tile([Y, M], f32)
                e_tiles = []
                for nt in range(NT):
                    e_ps = ps.tile([128, M], f32, tag=f"e{nt}")
                    with tc.tile_wait_until(DATA_READY_MS):
                        pk_consumers.append(
                            nc.tensor.matmul(
                                e_ps,
                                lhsT=pk_raw[
                                    0:3, 512 + nt * 128 : 512 + (nt + 1) * 128
                                ],
                                rhs=pk_raw[0:3, 0:512],
                                start=True,
                                stop=True,
                            )
                        )
                    e_tiles.append(e_ps)
                for nt in range(NT):
                    k_sb = sb.tile([128, M], bf16, tag=f"k{nt}")
                    nc.scalar.activation(
                        k_sb,
                        e_tiles[nt],
                        mybir.ActivationFunctionType.Exp,
                        bias=z1[:, 0:1],
                    )
                    if nt == 0:
                        # Filler matmul keeps the PE p-state ramp alive
                        # through the exp wait.
                        nc.tensor.matmul(
                            w_ps, lhsT=scr[:, 0:128], rhs=scr, start=True, stop=True
                        )
                    nc.tensor.matmul(
                        o_ps,
                        lhsT=zw_sb[:, nt * Y : (nt + 1) * Y],
                        rhs=k_sb,
                        start=(nt == 0),
                        stop=(nt == NT - 1),
                    )
                o_sb = sb.tile([Y, M], f32)
                nc.scalar.copy(o_sb, o_ps)
                nc.sync.dma_start(out=o[:], in_=o_sb)
        # Attach the input-DMA wait to every PE consumer of pk_raw, now that
        # the tile scheduler has run (it may have reordered them); later
        # waits are trivially satisfied.
        for mm in pk_consumers:
            mm.wait_op(dma_sem, 16, "sem-ge")
    _split_multi_waits(nc)
    return nc


def _get_nc():
    if "nc" not in _CACHE:
        _CACHE["nc"] = build_bass()
    return _CACHE["nc"]


def _in_maps_for_group(t, x, zw, s):
    """Build the 8 per-core input dicts for one sigma-group.

    zw: (B, N, Y) = z[:, :, group] @ W[:, group].T
    """
    import ml_dtypes

    in_maps = []
    for core in range(8):
        b, h = core // 2, core % 2
        xb = x[b, :, 0]
        pkm = np.zeros((4, PKW), np.float32)
        pkm[0, 0:512] = s * xb * xb
        pkm[1, 0:512] = 1.0
        pkm[2, 0:512] = -2.0 * s * xb
        for nt in range(NT):
            tb = t[b, h * NHALF + nt * 128 : h * NHALF + (nt + 1) * 128, 0]
            c0 = 512 + nt * 128
            pkm[0, c0 : c0 + 128] = 1.0
            pkm[1, c0 : c0 + 128] = s * tb * tb
            pkm[2, c0 : c0 + 128] = tb
            lo = h * NHALF + nt * 128
            pkm[nt * Y, 768:896][0:128] = zw[b, lo : lo + 128, 0]
            pkm[nt * Y + 1, 768:896][0:128] = zw[b, lo : lo + 128, 1]
        pkm[0:4, 896:900] = np.eye(4, dtype=np.float32)
        in_maps.append({"pk": pkm.astype(ml_dtypes.bfloat16)})
    return in_maps


def _run_group(t, x, zw, s, trace=False):
    from concourse.bass_utils import run_bass_kernel_spmd

    res = run_bass_kernel_spmd(
        _get_nc(),
        _in_maps_for_group(t, x, zw, s),
        core_ids=list(range(8)),
        trace=trace,
    )
    out = np.zeros((B, M, Y), np.float32)
    for b in range(B):
        acc = res.results[2 * b]["o"] + res.results[2 * b + 1]["o"]  # (Y, M)
        out[b] = acc.T
    return out, res


def kernel(**inputs):
    t = np.asarray(inputs["t"], np.float32)
    z = np.asarray(inputs["z"], np.float32)
    x = np.asarray(inputs["x"], np.float32)
    sigma = np.asarray(inputs["sigma"], np.float32)
    W = np.asarray(inputs["W"], np.float32)
    bias = np.asarray(inputs["b"], np.float32)

    trace = bool(_CACHE.pop("trace", False))
    out = np.zeros((B, M, Y), np.float32)
    if np.all(sigma == sigma[0]):
        s = -0.5 * float(np.exp(-2.0 * sigma[0]))
        zw = z @ W.T  # (B, N, Y)
        grp_out, res = _run_group(t, x, zw.astype(np.float32), s, trace=trace)
        out += grp_out
        _CACHE["last_results"] = res
    else:
        for val in np.unique(sigma):
            idx = np.nonzero(sigma == val)[0]
            zw = z[:, :, idx] @ W[:, idx].T
            s = -0.5 * float(np.exp(-2.0 * val))
            grp_out, res = _run_group(t, x, zw.astype(np.float32), s, trace=False)
            out += grp_out
    out += bias[None, None, :]
    return out
